# Initial kernel scaffold
#
"""Trainium2 Bass kernel for a DeepSpeed-style transformer encoder layer.

Strategy: data-parallel over 8 NeuronCores. Each core owns 1024 tokens
(half of one batch's sequence). K/V are computed redundantly for the full
2048-token sequence on each core (cheaper than a collective exchange), so
there is no cross-core communication at all. Odd cores receive their
sequence rolled by 1024 so one SPMD program serves all cores: "own" tokens
are always rows 0..1023 (attention is permutation-invariant over keys).

Compute is bf16 on the TensorEngine; LayerNorm statistics, PSUM
accumulation and residuals are fp32. Attention runs entirely in
transposed [feature, token] layout; the softmax denominator is obtained
for free by appending a ones-column to V (so the ctx matmul also produces
sum(probs)), and the division is applied via reciprocal + a K=1 broadcast
matmul.
"""

import contextlib
import ctypes
import os
import sys
import types

import numpy as np
import ml_dtypes

B, S, H = 4, 2048, 1024
HEADS, HD, DFF, P = 16, 64, 4096, 128
NCORES = 8
TOK = 2048          # k/v token domain per core (full sequence)
OWN = 1024          # query tokens per core
EPS = 1e-12
SCALE = 1.0 / 8.0   # 1/sqrt(HD)

_CACHE = {}
LAST_EXEC_NS = None


# ---------------------------------------------------------------- trace hook
def _install_trace_hook():
    """Recreate the antenv.axon_hooks NTFF profile hook missing from this
    image, so run_bass_kernel_spmd(trace=True) works (used by test.py)."""
    if "antenv.axon_hooks" in sys.modules:
        return
    so_path = "/opt/axon/libaxon_pjrt.so"

    def _make(so):
        try:
            lib = ctypes.CDLL(so)
        except OSError:
            return None
        if not hasattr(lib, "axon_start_nrt_profile"):
            return None
        lib.axon_start_nrt_profile.argtypes = [
            ctypes.POINTER(ctypes.c_int64), ctypes.c_size_t]
        lib.axon_start_nrt_profile.restype = ctypes.c_int64
        lib.axon_stop_nrt_profile.argtypes = [ctypes.c_char_p]
        lib.axon_stop_nrt_profile.restype = ctypes.c_int64

        @contextlib.contextmanager
        def _hook(output_dir, device_ids):
            import jax
            jax.devices()
            if device_ids:
                ids = (ctypes.c_int64 * len(device_ids))(*device_ids)
                rc = lib.axon_start_nrt_profile(ids, len(device_ids))
            else:
                rc = lib.axon_start_nrt_profile(None, 0)
            if rc != 0:
                raise RuntimeError(f"axon_start_nrt_profile rc={rc}")
            try:
                yield
            finally:
                n = lib.axon_stop_nrt_profile(str(output_dir).encode())
                print(f"profile: {n} file(s) -> {output_dir}", file=sys.stderr)

        return _hook

    hook = _make(so_path)
    mod = types.ModuleType("antenv.axon_hooks")
    mod.get_axon_ntff_profile_hook = lambda: hook
    mod.set_axon_ntff_profile_hook = lambda h: None
    sys.modules["antenv.axon_hooks"] = mod
    import concourse.bass_utils as bu
    bu.upload_artifacts = lambda tmpdir: tmpdir


# ---------------------------------------------------------------- IR builder
def _build(flags):
    import concourse.bass as bass
    import concourse.mybir as mybir
    import concourse.tile as tile
    from concourse import bacc
    from concourse.masks import make_identity

    dt = mybir.dt
    AF = mybir.ActivationFunctionType
    OP = mybir.AluOpType

    nc = bacc.Bacc("TRN2", target_bir_lowering=False, debug=False,
                   enable_asserts=False, num_devices=NCORES)

    x_d = nc.dram_tensor("x", [16, P, H], dt.float32, kind="ExternalInput").ap()
    wq_d = nc.dram_tensor("wq", [8, 8, P, P], dt.bfloat16, kind="ExternalInput").ap()
    wk_d = nc.dram_tensor("wk", [8, 8, P, P], dt.bfloat16, kind="ExternalInput").ap()
    wv_d = nc.dram_tensor("wv", [2, 8, P, 512], dt.bfloat16, kind="ExternalInput").ap()
    wo_d = nc.dram_tensor("wo", [8, 2, P, 512], dt.bfloat16, kind="ExternalInput").ap()
    w1_d = nc.dram_tensor("w1", [32, 8, P, P], dt.bfloat16, kind="ExternalInput").ap()
    w2_d = nc.dram_tensor("w2", [32, 2, P, 512], dt.bfloat16, kind="ExternalInput").ap()
    out_d = nc.dram_tensor("out", [8, P, H], dt.float32, kind="ExternalOutput").ap()

    opt_d = {}
    if flags["ln1"]:
        opt_d["nw"] = nc.dram_tensor("nw", [P, H], dt.float32, kind="ExternalInput").ap()
        opt_d["nb"] = nc.dram_tensor("nb", [P, H], dt.float32, kind="ExternalInput").ap()
    if flags["ln2"]:
        opt_d["anw"] = nc.dram_tensor("anw", [P, H], dt.float32, kind="ExternalInput").ap()
        opt_d["anb"] = nc.dram_tensor("anb", [P, H], dt.float32, kind="ExternalInput").ap()
    if flags["bqk"]:
        opt_d["bqk"] = nc.dram_tensor("bqk", [16, P, 1], dt.float32, kind="ExternalInput").ap()
    if flags["bv"]:
        opt_d["bv"] = nc.dram_tensor("bv", [2, P, 512], dt.float32, kind="ExternalInput").ap()
    if flags["bo"]:
        opt_d["bo"] = nc.dram_tensor("bo", [P, H], dt.float32, kind="ExternalInput").ap()
    if flags["b1"]:
        opt_d["b1"] = nc.dram_tensor("b1", [32, P, 1], dt.float32, kind="ExternalInput").ap()
    if flags["b2"]:
        opt_d["b2"] = nc.dram_tensor("b2", [P, H], dt.float32, kind="ExternalInput").ap()
    if flags["mask"]:
        opt_d["mask"] = nc.dram_tensor("mask", [P, 16], dt.float32, kind="ExternalInput").ap()

    with tile.TileContext(nc, pool_alloc_mode="queue") as tc:
        es = contextlib.ExitStack()
        with es:
            const = es.enter_context(tc.tile_pool(name="const", bufs=1))
            ident = const.tile([P, P], dt.bfloat16)
            make_identity(nc, ident)
            zero_c = const.tile([P, 1], dt.float32)
            nc.vector.memset(zero_c[:], 0.0)
            eps_c = const.tile([P, 1], dt.float32)
            nc.vector.memset(eps_c[:], EPS)
            ones1 = const.tile([1, 64], dt.float32)
            nc.vector.memset(ones1[:], 1.0)

            opt_sb = {}
            for k, ap in opt_d.items():
                t = const.tile(list(ap.shape), dt.float32, name=f"sb_{k}")
                nc.sync.dma_start(t[:], ap[:])
                opt_sb[k] = t

            # ---------------- resident activations (lifetime-managed) ----
            x_es = contextlib.ExitStack()
            xo_p = x_es.enter_context(tc.tile_pool(name="x_own", bufs=1))
            x_own = xo_p.tile([P, 8, H], dt.float32)

            qkv_es = contextlib.ExitStack()
            qT = qkv_es.enter_context(tc.tile_pool(name="qT", bufs=1)).tile(
                [P, 8, OWN], dt.bfloat16, name="qT_t")
            kT = qkv_es.enter_context(tc.tile_pool(name="kT", bufs=1)).tile(
                [P, 8, TOK], dt.bfloat16, name="kT_t")
            vplus = qkv_es.enter_context(tc.tile_pool(name="vplus", bufs=1)).tile(
                [P, 16, HEADS * 65], dt.bfloat16, name="vplus_t")
            # ones columns of vplus (col 64 of each head block)
            nc.vector.memset(
                vplus[:, :, :].rearrange("p a (h c) -> p a h c", h=HEADS)[:, :, :, 64:65],
                1.0)

            # ================= stage A: LN1 + transpose ==================
            a_es = contextlib.ExitStack()
            xlnT = a_es.enter_context(tc.tile_pool(name="xlnT", bufs=1)).tile(
                [P, 8, TOK], dt.bfloat16, name="xlnT_t")
            xs_p = a_es.enter_context(tc.tile_pool(name="xs", bufs=3))
            scr_p = a_es.enter_context(tc.tile_pool(name="scrA", bufs=2))
            st_p = a_es.enter_context(tc.tile_pool(name="stA", bufs=3))
            ptr_p = a_es.enter_context(tc.tile_pool(name="ptrA", bufs=2, space="PSUM"))

            def layer_norm(src_ap, dst_bf16, wk_sb, bk_sb):
                """LN over free axis (H=1024) of [128, 1024] src -> bf16 dst."""
                stats = st_p.tile([P, 12], dt.float32, tag="stats")
                mv = st_p.tile([P, 2], dt.float32, tag="mv")
                nc.vector.bn_stats(stats[:, 0:6], src_ap[:, 0:512])
                nc.vector.bn_stats(stats[:, 6:12], src_ap[:, 512:1024])
                nc.vector.bn_aggr(mv[:], stats[:])
                sig = st_p.tile([P, 1], dt.float32, tag="sig")
                nc.scalar.activation(sig[:], mv[:, 1:2], AF.Sqrt, bias=eps_c[:])
                rsig = st_p.tile([P, 1], dt.float32, tag="rsig")
                nc.vector.reciprocal(rsig[:], sig[:])
                nbias = st_p.tile([P, 1], dt.float32, tag="nbias")
                nc.vector.scalar_tensor_tensor(
                    nbias[:], mv[:, 0:1], -1.0, rsig[:], OP.mult, OP.mult)
                if wk_sb is None:
                    nc.scalar.activation(dst_bf16[:], src_ap[:], AF.Identity,
                                         bias=nbias[:], scale=rsig[:])
                else:
                    tmp = st_p.tile([P, H], dt.float32, tag="lntmp")
                    nc.scalar.activation(tmp[:], src_ap[:], AF.Identity,
                                         bias=nbias[:], scale=rsig[:])
                    if bk_sb is None:
                        nc.vector.tensor_tensor(dst_bf16[:], tmp[:], wk_sb[:], op=OP.mult)
                    else:
                        nc.vector.tensor_tensor(tmp[:], tmp[:], wk_sb[:], op=OP.mult)
                        nc.vector.tensor_tensor(dst_bf16[:], tmp[:], bk_sb[:], op=OP.add)

            def transpose_1024(src_bf16, dst, dst_col, tr_pool):
                """src [128tok, 1024feat] bf16 -> dst[:, hb, dst_col:+128] for hb 0..7."""
                for half in range(2):
                    pt = tr_pool.tile([P, 512], dt.bfloat16, tag="ptr")
                    for j in range(4):
                        hb = half * 4 + j
                        nc.tensor.transpose(
                            pt[:, j * P:(j + 1) * P],
                            src_bf16[:, hb * P:(hb + 1) * P], ident[:])
                    nc.vector.tensor_copy(
                        dst[:, half * 4:(half + 1) * 4, dst_col:dst_col + P],
                        pt[:].rearrange("p (a b) -> p a b", a=4))

            ln1_w = opt_sb.get("nw")
            ln1_b = opt_sb.get("nb")
            for t in range(16):
                if t < 8:
                    xt = x_own[:, t, :]
                else:
                    xt_t = xs_p.tile([P, H], dt.float32, tag="xs")
                    nc.sync.dma_start(xt_t[:], x_d[t])
                    xt = xt_t[:]
                if t < 8:
                    nc.sync.dma_start(x_own[:, t, :], x_d[t])
                xln = scr_p.tile([P, H], dt.bfloat16, tag="xln")
                layer_norm(xt, xln, ln1_w, ln1_b)
                transpose_1024(xln, xlnT, t * P, ptr_p)

            # ================= stage B: QKV projections ==================
            b_es = contextlib.ExitStack()
            wq_p = b_es.enter_context(tc.tile_pool(name="wq", bufs=2))
            wk_p = b_es.enter_context(tc.tile_pool(name="wk", bufs=2))
            wv_p = b_es.enter_context(tc.tile_pool(name="wv", bufs=2))
            pmm_p = b_es.enter_context(tc.tile_pool(name="pmmB", bufs=3, space="PSUM"))

            bqk = opt_sb.get("bqk")

            def qkv_feat_block(w_dram, mb, nchunks, dstT, bias_row):
                wt = (wq_p if w_dram is wq_d else wk_p).tile(
                    [P, 8, P], dt.bfloat16, tag="wqk")
                nc.sync.dma_start(wt[:], w_dram[mb].rearrange("a p b -> p a b"))
                for n in range(nchunks):
                    ps = pmm_p.tile([P, 512], dt.float32, tag="pmm")
                    for hb in range(8):
                        nc.tensor.matmul(ps[:], wt[:, hb, :],
                                         xlnT[:, hb, n * 512:(n + 1) * 512],
                                         start=(hb == 0), stop=(hb == 7))
                    dst = dstT[:, mb, n * 512:(n + 1) * 512]
                    if bias_row is None:
                        nc.vector.tensor_copy(dst, ps[:])
                    else:
                        nc.vector.tensor_scalar(
                            out=dst, in0=ps[:], scalar1=bias_row, scalar2=None,
                            op0=OP.add)

            for mb in range(8):
                qkv_feat_block(wq_d, mb, 2, qT, bqk[:, mb, :] if bqk is not None else None)
            for mb in range(8):
                qkv_feat_block(wk_d, mb, 4, kT, bqk[:, 8 + mb, :] if bqk is not None else None)

            bv = opt_sb.get("bv")
            for nb in range(2):
                wv_t = wv_p.tile([P, 8, 512], dt.bfloat16, tag="wv")
                nc.sync.dma_start(wv_t[:], wv_d[nb].rearrange("a p b -> p a b"))
                for tt in range(16):
                    ps = pmm_p.tile([P, 512], dt.float32, tag="pmm")
                    for hb in range(8):
                        nc.tensor.matmul(ps[:], xlnT[:, hb, tt * P:(tt + 1) * P],
                                         wv_t[:, hb, :],
                                         start=(hb == 0), stop=(hb == 7))
                    dst = vplus[:, tt, :].rearrange(
                        "p (h c) -> p h c", h=HEADS)[:, nb * 8:(nb + 1) * 8, 0:64]
                    src = ps[:].rearrange("p (h c) -> p h c", h=8)
                    if bv is None:
                        nc.vector.tensor_copy(dst, src)
                    else:
                        nc.vector.tensor_tensor(
                            dst, src,
                            bv[:, nb, :].rearrange("p (h c) -> p h c", h=8), op=OP.add)

            a_es.close()   # free xlnT + stage-A scratch
            b_es.close()

            # ================= stage C: attention ========================
            c_es = contextlib.ExitStack()
            ctxT = c_es.enter_context(tc.tile_pool(name="ctxT", bufs=1)).tile(
                [P, 8, OWN], dt.bfloat16, name="ctxT_t")
            ps_s = c_es.enter_context(tc.tile_pool(name="ps_s", bufs=2, space="PSUM"))
            ps_c = c_es.enter_context(tc.tile_pool(name="ps_c", bufs=2, space="PSUM"))
            ps_b = c_es.enter_context(tc.tile_pool(name="ps_b", bufs=2, space="PSUM"))
            pr_p = c_es.enter_context(tc.tile_pool(name="probs", bufs=4))
            rr_p = c_es.enter_context(tc.tile_pool(name="rrow", bufs=2))
            rb_p = c_es.enter_context(tc.tile_pool(name="rbc", bufs=2))

            mask_sb = opt_sb.get("mask")
            for h in range(HEADS):
                hb = h // 2
                hp = (h % 2) * 64
                for qb in range(2):
                    qs = qb * 512
                    pc = ps_c.tile([P, 512], dt.float32, tag="psc")
                    for kc in range(16):
                        pss = ps_s.tile([P, 512], dt.float32, tag="pss")
                        nc.tensor.matmul(
                            pss[:],
                            kT[hp:hp + 64, hb, kc * P:(kc + 1) * P],
                            qT[hp:hp + 64, hb, qs:qs + 512],
                            start=True, stop=True)
                        prob = pr_p.tile([P, 512], dt.bfloat16, tag="prob")
                        bias = (mask_sb[:, kc:kc + 1] if mask_sb is not None
                                else zero_c[:])
                        nc.scalar.activation(prob[:], pss[:], AF.Exp,
                                             bias=bias, scale=SCALE)
                        nc.tensor.matmul(
                            pc[0:65, :],
                            vplus[:, kc, h * 65:(h + 1) * 65],
                            prob[:],
                            start=(kc == 0), stop=(kc == 15))
                    rrow = rr_p.tile([1, 512], dt.float32, tag="rrow")
                    nc.vector.reciprocal(rrow[:], pc[64:65, :])
                    pb = ps_b.tile([64, 512], dt.float32, tag="psb")
                    nc.tensor.matmul(pb[:], ones1[:], rrow[:], start=True, stop=True)
                    rb = rb_p.tile([64, 512], dt.float32, tag="rb")
                    nc.scalar.copy(rb[:], pb[:])
                    nc.vector.tensor_tensor(
                        ctxT[hp:hp + 64, hb, qs:qs + 512],
                        pc[0:64, :], rb[:], op=OP.mult)

            # ================= stage D: proj + residual + LN2 ============
            d_es = contextlib.ExitStack()
            ao_p = d_es.enter_context(tc.tile_pool(name="attn_out", bufs=1))
            attn_out = ao_p.tile([P, 8, H], dt.bfloat16)
            yl_p = d_es.enter_context(tc.tile_pool(name="ylnT", bufs=1))
            ylnT = yl_p.tile([P, 8, OWN], dt.bfloat16)
            wo_p = d_es.enter_context(tc.tile_pool(name="wo", bufs=1))
            wo_sb = wo_p.tile([P, 8, 1024], dt.bfloat16)
            for cb in range(8):
                nc.sync.dma_start(
                    wo_sb[:, cb, :].rearrange("p (a b) -> p a b", a=2),
                    wo_d[cb].rearrange("a p b -> p a b"))
            pmm2_p = d_es.enter_context(tc.tile_pool(name="pmmD", bufs=3, space="PSUM"))
            scr2_p = d_es.enter_context(tc.tile_pool(name="scrD", bufs=2))
            st2_p = d_es.enter_context(tc.tile_pool(name="stD", bufs=3))
            ptr2_p = d_es.enter_context(tc.tile_pool(name="ptrD", bufs=2, space="PSUM"))

            bo = opt_sb.get("bo")
            for mb in range(8):
                for nb in range(2):
                    ps = pmm2_p.tile([P, 512], dt.float32, tag="pmm2")
                    for cb in range(8):
                        nc.tensor.matmul(
                            ps[:], ctxT[:, cb, mb * P:(mb + 1) * P],
                            wo_sb[:, cb, nb * 512:(nb + 1) * 512],
                            start=(cb == 0), stop=(cb == 7))
                    dst = attn_out[:, mb, nb * 512:(nb + 1) * 512]
                    xs = x_own[:, mb, nb * 512:(nb + 1) * 512]
                    if bo is None:
                        nc.vector.tensor_tensor(dst, ps[:], xs, op=OP.add)
                    else:
                        tmp = scr2_p.tile([P, 512], dt.float32, tag="botmp")
                        nc.vector.tensor_tensor(tmp[:], ps[:], xs, op=OP.add)
                        nc.vector.tensor_tensor(
                            dst, tmp[:], bo[:, nb * 512:(nb + 1) * 512], op=OP.add)

            ln2_w = opt_sb.get("anw")
            ln2_b = opt_sb.get("anb")
            _saved = st_p, scr_p
            for mb in range(8):
                stats = st2_p.tile([P, 12], dt.float32, tag="stats2")
                mv = st2_p.tile([P, 2], dt.float32, tag="mv2")
                nc.vector.bn_stats(stats[:, 0:6], attn_out[:, mb, 0:512])
                nc.vector.bn_stats(stats[:, 6:12], attn_out[:, mb, 512:1024])
                nc.vector.bn_aggr(mv[:], stats[:])
                sig = st2_p.tile([P, 1], dt.float32, tag="sig2")
                nc.scalar.activation(sig[:], mv[:, 1:2], AF.Sqrt, bias=eps_c[:])
                rsig = st2_p.tile([P, 1], dt.float32, tag="rsig2")
                nc.vector.reciprocal(rsig[:], sig[:])
                nbias = st2_p.tile([P, 1], dt.float32, tag="nbias2")
                nc.vector.scalar_tensor_tensor(
                    nbias[:], mv[:, 0:1], -1.0, rsig[:], OP.mult, OP.mult)
                yln = scr2_p.tile([P, H], dt.bfloat16, tag="yln")
                if ln2_w is None:
                    nc.scalar.activation(yln[:], attn_out[:, mb, :], AF.Identity,
                                         bias=nbias[:], scale=rsig[:])
                else:
                    tmp = st2_p.tile([P, H], dt.float32, tag="lntmp2")
                    nc.scalar.activation(tmp[:], attn_out[:, mb, :], AF.Identity,
                                         bias=nbias[:], scale=rsig[:])
                    if ln2_b is None:
                        nc.vector.tensor_tensor(yln[:], tmp[:], ln2_w[:], op=OP.mult)
                    else:
                        nc.vector.tensor_tensor(tmp[:], tmp[:], ln2_w[:], op=OP.mult)
                        nc.vector.tensor_tensor(yln[:], tmp[:], ln2_b[:], op=OP.add)
                transpose_1024(yln, ylnT, mb * P, ptr2_p)

            c_es.close()
            qkv_es.close()
            x_es.close()
            wo_p  # keep reference; freed with d_es below

            # ================= stage E: FFN ==============================
            e_es = contextlib.ExitStack()
            hT_p = e_es.enter_context(tc.tile_pool(name="hT", bufs=1))
            hT = hT_p.tile([P, 32, OWN], dt.bfloat16)
            w1_p = e_es.enter_context(tc.tile_pool(name="w1", bufs=3))
            w2_p = e_es.enter_context(tc.tile_pool(name="w2", bufs=1))
            w2_sb = w2_p.tile([P, 32, 1024], dt.bfloat16)
            for fb in range(32):
                nc.sync.dma_start(
                    w2_sb[:, fb, :].rearrange("p (a b) -> p a b", a=2),
                    w2_d[fb].rearrange("a p b -> p a b"))
            pmm3_p = e_es.enter_context(tc.tile_pool(name="pmmE", bufs=3, space="PSUM"))
            out_p = e_es.enter_context(tc.tile_pool(name="outp", bufs=2))

            b1 = opt_sb.get("b1")
            for fb in range(32):
                w1_t = w1_p.tile([P, 8, P], dt.bfloat16, tag="w1t")
                nc.sync.dma_start(w1_t[:], w1_d[fb].rearrange("a p b -> p a b"))
                for qb in range(2):
                    ps = pmm3_p.tile([P, 512], dt.float32, tag="pmm3")
                    for hb in range(8):
                        nc.tensor.matmul(
                            ps[:], w1_t[:, hb, :],
                            ylnT[:, hb, qb * 512:(qb + 1) * 512],
                            start=(hb == 0), stop=(hb == 7))
                    bias = b1[:, fb, :] if b1 is not None else zero_c[:]
                    nc.scalar.activation(hT[:, fb, qb * 512:(qb + 1) * 512],
                                         ps[:], AF.Gelu, bias=bias)

            b2 = opt_sb.get("b2")
            for mb in range(8):
                outt = out_p.tile([P, H], dt.float32, tag="outt")
                for nb in range(2):
                    ps = pmm3_p.tile([P, 512], dt.float32, tag="pmm3")
                    for fb in range(32):
                        nc.tensor.matmul(
                            ps[:], hT[:, fb, mb * P:(mb + 1) * P],
                            w2_sb[:, fb, nb * 512:(nb + 1) * 512],
                            start=(fb == 0), stop=(fb == 31))
                    dst = outt[:, nb * 512:(nb + 1) * 512]
                    res = attn_out[:, mb, nb * 512:(nb + 1) * 512]
                    if b2 is None:
                        nc.vector.tensor_tensor(dst, ps[:], res, op=OP.add)
                    else:
                        nc.vector.tensor_tensor(dst, ps[:], res, op=OP.add)
                        nc.vector.tensor_tensor(
                            dst, dst, b2[:, nb * 512:(nb + 1) * 512], op=OP.add)
                nc.sync.dma_start(out_d[mb], outt[:])

            d_es.close()
            e_es.close()

    nc.compile()
    return nc


# ---------------------------------------------------------------- host side
def _prep_weights(qkv_w, attn_ow, inter_w, output_w):
    bf = ml_dtypes.bfloat16
    wq = qkv_w[:, 0:1024].reshape(8, P, 8, P).transpose(2, 0, 1, 3).astype(bf)
    wk = qkv_w[:, 1024:2048].reshape(8, P, 8, P).transpose(2, 0, 1, 3).astype(bf)
    wv = qkv_w[:, 2048:3072].reshape(8, P, 2, 512).transpose(2, 0, 1, 3).astype(bf)
    wo = attn_ow.reshape(8, P, 2, 512).transpose(0, 2, 1, 3).astype(bf)
    w1 = inter_w.reshape(8, P, 32, P).transpose(2, 0, 1, 3).astype(bf)
    w2 = output_w.reshape(32, P, 2, 512).transpose(0, 2, 1, 3).astype(bf)
    return (np.ascontiguousarray(wq), np.ascontiguousarray(wk),
            np.ascontiguousarray(wv), np.ascontiguousarray(wo),
            np.ascontiguousarray(w1), np.ascontiguousarray(w2))


def kernel(input, input_mask, norm_w, norm_b, qkv_w, qkv_b, attn_ow, attn_ob,
           attn_nw, attn_nb, inter_w, inter_b, output_w, output_b):
    global LAST_EXEC_NS
    _install_trace_hook()
    from concourse.bass_utils import run_bass_kernel_spmd

    input = np.asarray(input, dtype=np.float32)
    input_mask = np.asarray(input_mask, dtype=np.float32)
    f32 = lambda a: np.asarray(a, dtype=np.float32)
    norm_w, norm_b = f32(norm_w), f32(norm_b)
    qkv_b, attn_ob = f32(qkv_b), f32(attn_ob)
    attn_nw, attn_nb = f32(attn_nw), f32(attn_nb)
    inter_b, output_b = f32(inter_b), f32(output_b)

    flags = {
        "ln1": not (np.all(norm_w == 1.0) and np.all(norm_b == 0.0)),
        "ln2": not (np.all(attn_nw == 1.0) and np.all(attn_nb == 0.0)),
        "bqk": bool(np.any(qkv_b[0:2048] != 0.0)),
        "bv": bool(np.any(qkv_b[2048:3072] != 0.0)),
        "bo": bool(np.any(attn_ob != 0.0)),
        "b1": bool(np.any(inter_b != 0.0)),
        "b2": bool(np.any(output_b != 0.0)),
        "mask": bool(np.any(input_mask != 0.0)),
    }
    key = tuple(sorted(flags.items()))
    if key not in _CACHE:
        _CACHE[key] = _build(flags)
    nc = _CACHE[key]

    wq, wk, wv, wo, w1, w2 = _prep_weights(
        f32(qkv_w), f32(attn_ow), f32(inter_w), f32(output_w))

    common = {"wq": wq, "wk": wk, "wv": wv, "wo": wo, "w1": w1, "w2": w2}
    bcast = lambda v: np.ascontiguousarray(
        np.broadcast_to(v.reshape(1, H), (P, H)).astype(np.float32))
    if flags["ln1"]:
        common["nw"] = bcast(norm_w); common["nb"] = bcast(norm_b)
    if flags["ln2"]:
        common["anw"] = bcast(attn_nw); common["anb"] = bcast(attn_nb)
    if flags["bqk"]:
        common["bqk"] = np.ascontiguousarray(
            qkv_b[0:2048].reshape(16, P, 1).astype(np.float32))
    if flags["bv"]:
        common["bv"] = np.ascontiguousarray(
            qkv_b[2048:3072].reshape(2, 512)[:, None, :].repeat(P, 1).astype(np.float32))
    if flags["bo"]:
        common["bo"] = bcast(attn_ob)
    if flags["b1"]:
        common["b1"] = np.ascontiguousarray(
            inter_b.reshape(32, P, 1).astype(np.float32))
    if flags["b2"]:
        common["b2"] = bcast(output_b)

    in_maps = []
    for c in range(NCORES):
        b, r = c // 2, (c % 2) * OWN
        xb = np.roll(input[b], -r, axis=0) if r else input[b]
        m = dict(common)
        m["x"] = np.ascontiguousarray(xb.reshape(16, P, H))
        if flags["mask"]:
            mk = input_mask[b, 0, 0]
            mk = np.roll(mk, -r) if r else mk
            m["mask"] = np.ascontiguousarray(
                np.broadcast_to(mk.reshape(16, P).T.reshape(1, ...)
                                if False else mk.reshape(16, P).T, (P, 16)
                                ).astype(np.float32)) if False else \
                np.ascontiguousarray(mk.reshape(16, P).T.copy())
        in_maps.append(m)

    trace = bool(os.environ.get("BASS_TRACE"))
    res = run_bass_kernel_spmd(nc, in_maps, list(range(NCORES)), trace=trace)
    LAST_EXEC_NS = res.exec_time_ns
    if res.exec_time_ns is not None:
        print(f"HW exec time: {res.exec_time_ns} ns")

    out = np.empty((B, S, H), dtype=np.float32)
    for c in range(NCORES):
        b, r = c // 2, (c % 2) * OWN
        out[b, r:r + OWN] = res.results[c]["out"].reshape(OWN, H)
    return out


# revision 9
# speedup vs baseline: 1.5147x; 1.5147x over previous
"""Trainium2 Bass kernel for a DeepSpeed-style transformer encoder layer.

Strategy: data-parallel over 8 NeuronCores. Each core owns 1024 tokens
(half of one batch's sequence). K/V are computed redundantly for the full
2048-token sequence on each core (cheaper than a collective exchange), so
there is no cross-core communication at all. Odd cores receive their
sequence rolled by 1024 so one SPMD program serves all cores: "own" tokens
are always rows 0..1023 (attention is permutation-invariant over keys).

Compute is bf16 on the TensorEngine; LayerNorm statistics, PSUM
accumulation and residuals are fp32. Attention runs entirely in
transposed [feature, token] layout; the softmax denominator is obtained
for free by appending a ones-column to V (so the ctx matmul also produces
sum(probs)), and the division is applied via reciprocal + a K=1 broadcast
matmul.
"""

import contextlib
import ctypes
import os
import sys
import types

import numpy as np
import ml_dtypes

B, S, H = 4, 2048, 1024
HEADS, HD, DFF, P = 16, 64, 4096, 128
NCORES = 8
TOK = 2048          # k/v token domain per core (full sequence)
OWN = 1024          # query tokens per core
EPS = 1e-12
SCALE = 1.0 / 8.0   # 1/sqrt(HD)

_CACHE = {}
LAST_EXEC_NS = None


# ---------------------------------------------------------------- trace hook
def _install_trace_hook():
    """Recreate the antenv.axon_hooks NTFF profile hook missing from this
    image, so run_bass_kernel_spmd(trace=True) works (used by test.py)."""
    if "antenv.axon_hooks" in sys.modules:
        return
    so_path = "/opt/axon/libaxon_pjrt.so"

    def _make(so):
        try:
            lib = ctypes.CDLL(so)
        except OSError:
            return None
        if not hasattr(lib, "axon_start_nrt_profile"):
            return None
        lib.axon_start_nrt_profile.argtypes = [
            ctypes.POINTER(ctypes.c_int64), ctypes.c_size_t]
        lib.axon_start_nrt_profile.restype = ctypes.c_int64
        lib.axon_stop_nrt_profile.argtypes = [ctypes.c_char_p]
        lib.axon_stop_nrt_profile.restype = ctypes.c_int64

        @contextlib.contextmanager
        def _hook(output_dir, device_ids):
            import jax
            jax.devices()
            if device_ids:
                ids = (ctypes.c_int64 * len(device_ids))(*device_ids)
                rc = lib.axon_start_nrt_profile(ids, len(device_ids))
            else:
                rc = lib.axon_start_nrt_profile(None, 0)
            if rc != 0:
                raise RuntimeError(f"axon_start_nrt_profile rc={rc}")
            try:
                yield
            finally:
                n = lib.axon_stop_nrt_profile(str(output_dir).encode())
                print(f"profile: {n} file(s) -> {output_dir}", file=sys.stderr)

        return _hook

    hook = _make(so_path)
    mod = types.ModuleType("antenv.axon_hooks")
    mod.get_axon_ntff_profile_hook = lambda: hook
    mod.set_axon_ntff_profile_hook = lambda h: None
    sys.modules["antenv.axon_hooks"] = mod
    import concourse.bass_utils as bu
    bu.upload_artifacts = lambda tmpdir: tmpdir


# ---------------------------------------------------------------- IR builder
def _build(flags):
    import concourse.bass as bass
    import concourse.mybir as mybir
    import concourse.tile as tile
    from concourse import bacc
    from concourse.masks import make_identity

    dt = mybir.dt
    AF = mybir.ActivationFunctionType
    OP = mybir.AluOpType

    nc = bacc.Bacc("TRN2", target_bir_lowering=False, debug=False,
                   enable_asserts=False, num_devices=NCORES)

    x_d = nc.dram_tensor("x", [16, P, H], dt.float32, kind="ExternalInput").ap()
    wq_d = nc.dram_tensor("wq", [8, 8, P, P], dt.bfloat16, kind="ExternalInput").ap()
    wk_d = nc.dram_tensor("wk", [8, 8, P, P], dt.bfloat16, kind="ExternalInput").ap()
    wv_d = nc.dram_tensor("wv", [2, 8, P, 512], dt.bfloat16, kind="ExternalInput").ap()
    wo_d = nc.dram_tensor("wo", [8, 2, P, 512], dt.bfloat16, kind="ExternalInput").ap()
    w1_d = nc.dram_tensor("w1", [32, 8, P, P], dt.bfloat16, kind="ExternalInput").ap()
    w2_d = nc.dram_tensor("w2", [32, 2, P, 512], dt.bfloat16, kind="ExternalInput").ap()
    out_d = nc.dram_tensor("out", [8, P, H], dt.float32, kind="ExternalOutput").ap()

    opt_d = {}
    if flags["ln1"]:
        opt_d["nw"] = nc.dram_tensor("nw", [P, H], dt.float32, kind="ExternalInput").ap()
        opt_d["nb"] = nc.dram_tensor("nb", [P, H], dt.float32, kind="ExternalInput").ap()
    if flags["ln2"]:
        opt_d["anw"] = nc.dram_tensor("anw", [P, H], dt.float32, kind="ExternalInput").ap()
        opt_d["anb"] = nc.dram_tensor("anb", [P, H], dt.float32, kind="ExternalInput").ap()
    if flags["bqk"]:
        opt_d["bqk"] = nc.dram_tensor("bqk", [16, P, 1], dt.float32, kind="ExternalInput").ap()
    if flags["bv"]:
        opt_d["bv"] = nc.dram_tensor("bv", [2, P, 512], dt.float32, kind="ExternalInput").ap()
    if flags["bo"]:
        opt_d["bo"] = nc.dram_tensor("bo", [P, H], dt.float32, kind="ExternalInput").ap()
    if flags["b1"]:
        opt_d["b1"] = nc.dram_tensor("b1", [32, P, 1], dt.float32, kind="ExternalInput").ap()
    if flags["b2"]:
        opt_d["b2"] = nc.dram_tensor("b2", [P, H], dt.float32, kind="ExternalInput").ap()
    if flags["mask"]:
        opt_d["mask"] = nc.dram_tensor("mask", [P, 16], dt.float32, kind="ExternalInput").ap()

    with tile.TileContext(nc) as tc:
        es = contextlib.ExitStack()
        with es:
            const = es.enter_context(tc.tile_pool(name="const", bufs=1))
            ident = const.tile([P, P], dt.bfloat16)
            make_identity(nc, ident)
            zero_c = const.tile([P, 1], dt.float32)
            nc.vector.memset(zero_c[:], 0.0)
            eps_c = const.tile([P, 1], dt.float32)
            nc.vector.memset(eps_c[:], EPS)
            ones1 = const.tile([1, 64], dt.float32)
            nc.vector.memset(ones1[:], 1.0)

            ps_mm = es.enter_context(tc.tile_pool(name="ps_mm", bufs=3, space="PSUM"))
            ps_tr = es.enter_context(tc.tile_pool(name="ps_tr", bufs=2, space="PSUM"))
            ps_c = es.enter_context(tc.tile_pool(name="ps_c", bufs=2, space="PSUM"))
            ps_b = es.enter_context(tc.tile_pool(name="ps_b", bufs=1, space="PSUM"))

            opt_sb = {}
            for k, ap in opt_d.items():
                t = const.tile(list(ap.shape), dt.float32, name=f"sb_{k}")
                nc.sync.dma_start(t[:], ap[:])
                opt_sb[k] = t

            # -------- resident activations (left side, strict LIFO) ------
            x_es = contextlib.ExitStack()
            xo_p = x_es.enter_context(tc.tile_pool(name="x_own", bufs=1, side="left"))
            x_own = xo_p.tile([P, 8, H], dt.float32)

            qkv_es = contextlib.ExitStack()
            qT = qkv_es.enter_context(tc.tile_pool(name="qT", bufs=1, side="left")).tile(
                [P, HEADS, OWN], dt.bfloat16, name="qT_t")
            # per-head q lives at its natural 64 partition rows; the other 64
            # rows stay zero so score matmuls can run full-K (keeps PE HAM warm)
            nc.vector.memset(qT[:], 0.0)
            kT = qkv_es.enter_context(tc.tile_pool(name="kT", bufs=1, side="left")).tile(
                [P, 8, TOK], dt.bfloat16, name="kT_t")
            vplus = qkv_es.enter_context(tc.tile_pool(name="vplus", bufs=1, side="left")).tile(
                [P, 16, HEADS * 65], dt.bfloat16, name="vplus_t")
            # ones columns of vplus (col 64 of each head block)
            nc.vector.memset(
                vplus[:, :, :].rearrange("p a (h c) -> p a h c", h=HEADS)[:, :, :, 64:65],
                1.0)

            # ================= stage A: LN1 + transpose ==================
            a_es = contextlib.ExitStack()
            xlnT = a_es.enter_context(tc.tile_pool(name="xlnT", bufs=1, side="right")).tile(
                [P, 8, TOK], dt.bfloat16, name="xlnT_t")
            xs_p = a_es.enter_context(tc.tile_pool(name="xs", bufs=3, side="right"))
            scr_p = a_es.enter_context(tc.tile_pool(name="scrA", bufs=2, side="right"))
            st_p = a_es.enter_context(tc.tile_pool(name="stA", bufs=3, side="right"))

            def layer_norm(src_ap, dst_bf16, wk_sb, bk_sb):
                """LN over free axis (H=1024) of [128, 1024] src -> bf16 dst."""
                stats = st_p.tile([P, 12], dt.float32, tag="stats")
                mv = st_p.tile([P, 2], dt.float32, tag="mv")
                nc.vector.bn_stats(stats[:, 0:6], src_ap[:, 0:512])
                nc.vector.bn_stats(stats[:, 6:12], src_ap[:, 512:1024])
                nc.vector.bn_aggr(mv[:], stats[:])
                sig = st_p.tile([P, 1], dt.float32, tag="sig")
                nc.scalar.activation(sig[:], mv[:, 1:2], AF.Sqrt, bias=eps_c[:])
                rsig = st_p.tile([P, 1], dt.float32, tag="rsig")
                nc.vector.reciprocal(rsig[:], sig[:])
                nbias = st_p.tile([P, 1], dt.float32, tag="nbias")
                nc.vector.scalar_tensor_tensor(
                    nbias[:], mv[:, 0:1], -1.0, rsig[:], OP.mult, OP.mult)
                if wk_sb is None:
                    nc.scalar.activation(dst_bf16[:], src_ap[:], AF.Identity,
                                         bias=nbias[:], scale=rsig[:])
                else:
                    tmp = st_p.tile([P, H], dt.float32, tag="lntmp")
                    nc.scalar.activation(tmp[:], src_ap[:], AF.Identity,
                                         bias=nbias[:], scale=rsig[:])
                    if bk_sb is None:
                        nc.vector.tensor_tensor(dst_bf16[:], tmp[:], wk_sb[:], op=OP.mult)
                    else:
                        nc.vector.tensor_tensor(tmp[:], tmp[:], wk_sb[:], op=OP.mult)
                        nc.vector.tensor_tensor(dst_bf16[:], tmp[:], bk_sb[:], op=OP.add)

            def transpose_1024(src_bf16, dst, dst_col, tr_pool):
                """src [128tok, 1024feat] bf16 -> dst[:, hb, dst_col:+128] for hb 0..7."""
                for half in range(2):
                    pt = tr_pool.tile([P, 512], dt.bfloat16, tag="ptr")
                    for j in range(4):
                        hb = half * 4 + j
                        nc.tensor.transpose(
                            pt[:, j * P:(j + 1) * P],
                            src_bf16[:, hb * P:(hb + 1) * P], ident[:])
                    nc.vector.tensor_copy(
                        dst[:, half * 4:(half + 1) * 4, dst_col:dst_col + P],
                        pt[:].rearrange("p (a b) -> p a b", a=4))

            ln1_w = opt_sb.get("nw")
            ln1_b = opt_sb.get("nb")
            for t in range(16):
                if t < 8:
                    xt = x_own[:, t, :]
                else:
                    xt_t = xs_p.tile([P, H], dt.float32, tag="xs")
                    nc.sync.dma_start(xt_t[:], x_d[t])
                    xt = xt_t[:]
                if t < 8:
                    nc.sync.dma_start(x_own[:, t, :], x_d[t])
                xln = scr_p.tile([P, H], dt.bfloat16, tag="xln")
                layer_norm(xt, xln, ln1_w, ln1_b)
                transpose_1024(xln, xlnT, t * P, ps_tr)

            # ================= stage B: QKV projections ==================
            wq_p = a_es.enter_context(tc.tile_pool(name="wq", bufs=2, side="right"))
            wk_p = a_es.enter_context(tc.tile_pool(name="wk", bufs=2, side="right"))
            wv_p = a_es.enter_context(tc.tile_pool(name="wv", bufs=2, side="right"))

            bqk = opt_sb.get("bqk")

            def qkv_feat_block(w_dram, mb, nchunks, dstT, bias_row, split_heads=False):
                wt = (wq_p if w_dram is wq_d else wk_p).tile(
                    [P, 8, P], dt.bfloat16, tag="wqk")
                nc.sync.dma_start(wt[:], w_dram[mb].rearrange("a p b -> p a b"))
                for n in range(nchunks):
                    ps = ps_mm.tile([P, 512], dt.float32, tag="pmm")
                    for hb in range(8):
                        nc.tensor.matmul(ps[:], wt[:, hb, :],
                                         xlnT[:, hb, n * 512:(n + 1) * 512],
                                         start=(hb == 0), stop=(hb == 7))
                    ns = slice(n * 512, (n + 1) * 512)
                    if split_heads:
                        dsts = [dstT[0:64, 2 * mb, ns], dstT[64:128, 2 * mb + 1, ns]]
                        srcs = [ps[0:64, :], ps[64:128, :]]
                    else:
                        dsts = [dstT[:, mb, ns]]
                        srcs = [ps[:]]
                    for dst, sp in zip(dsts, srcs):
                        if bias_row is None:
                            nc.vector.tensor_copy(dst, sp)
                        else:
                            nc.vector.tensor_scalar(dst, sp, bias_row, None, OP.add)

            for mb in range(8):
                qkv_feat_block(wq_d, mb, 2, qT, bqk[:, mb, :] if bqk is not None else None, split_heads=True)
            for mb in range(8):
                qkv_feat_block(wk_d, mb, 4, kT, bqk[:, 8 + mb, :] if bqk is not None else None)

            bv = opt_sb.get("bv")
            for nb in range(2):
                wv_t = wv_p.tile([P, 8, 512], dt.bfloat16, tag="wv")
                nc.sync.dma_start(wv_t[:], wv_d[nb].rearrange("a p b -> p a b"))
                for tt in range(16):
                    ps = ps_mm.tile([P, 512], dt.float32, tag="pmm")
                    for hb in range(8):
                        nc.tensor.matmul(ps[:], xlnT[:, hb, tt * P:(tt + 1) * P],
                                         wv_t[:, hb, :],
                                         start=(hb == 0), stop=(hb == 7))
                    dst = vplus[:, tt, :].rearrange(
                        "p (h c) -> p h c", h=HEADS)[:, nb * 8:(nb + 1) * 8, 0:64]
                    src = ps[:].rearrange("p (h c) -> p h c", h=8)
                    if bv is None:
                        nc.vector.tensor_copy(dst, src)
                    else:
                        nc.vector.tensor_tensor(
                            dst, src,
                            bv[:, nb, :].rearrange("p (h c) -> p h c", h=8), op=OP.add)

            a_es.close()   # free xlnT, stage-A scratch, qkv weight pools

            # ================= stage C: attention ========================
            ctx_es = contextlib.ExitStack()
            ctxT = ctx_es.enter_context(tc.tile_pool(name="ctxT", bufs=1, side="left")).tile(
                [P, 8, OWN], dt.bfloat16, name="ctxT_t")
            c_es = contextlib.ExitStack()
            pr_p = c_es.enter_context(tc.tile_pool(name="probs", bufs=4, side="right"))
            rr_p = c_es.enter_context(tc.tile_pool(name="rrow", bufs=2, side="right"))
            rb_p = c_es.enter_context(tc.tile_pool(name="rbc", bufs=2, side="right"))

            mask_sb = opt_sb.get("mask")
            for h in range(HEADS):
                hb = h // 2
                hp = (h % 2) * 64
                for qb in range(2):
                    qs = qb * 512
                    pc = ps_c.tile([P, 512], dt.float32, tag="psc")
                    for kc in range(16):
                        pss = ps_mm.tile([P, 512], dt.float32, tag="pmm")
                        nc.tensor.matmul(
                            pss[:],
                            kT[:, hb, kc * P:(kc + 1) * P],
                            qT[:, h, qs:qs + 512],
                            start=True, stop=True)
                        prob = pr_p.tile([P, 512], dt.bfloat16, tag="prob")
                        bias = (mask_sb[:, kc:kc + 1] if mask_sb is not None
                                else zero_c[:])
                        nc.scalar.activation(prob[:], pss[:], AF.Exp,
                                             bias=bias, scale=SCALE)
                        nc.tensor.matmul(
                            pc[0:65, :],
                            vplus[:, kc, h * 65:(h + 1) * 65],
                            prob[:],
                            start=(kc == 0), stop=(kc == 15))
                    rrow = rr_p.tile([1, 512], dt.float32, tag="rrow")
                    nc.vector.reciprocal(rrow[:], pc[64:65, :])
                    rb = rb_p.tile([64, 512], dt.float32, tag="rb")
                    nc.gpsimd.partition_broadcast(rb[:], rrow[:], channels=64)
                    nc.vector.tensor_tensor(
                        ctxT[hp:hp + 64, hb, qs:qs + 512],
                        pc[0:64, :], rb[:], op=OP.mult)

            # ================= stage D: proj + residual + LN2 ============
            c_es.close()   # free probs/rrow/rb scratch

            dkeep_es = contextlib.ExitStack()
            ao_p = dkeep_es.enter_context(tc.tile_pool(name="attn_out", bufs=1, side="right"))
            attn_out = ao_p.tile([P, 8, H], dt.bfloat16)
            yl_p = dkeep_es.enter_context(tc.tile_pool(name="ylnT", bufs=1, side="right"))
            ylnT = yl_p.tile([P, 8, OWN], dt.bfloat16)
            d_es = contextlib.ExitStack()
            wo_p = d_es.enter_context(tc.tile_pool(name="wo", bufs=1, side="right"))
            wo_sb = wo_p.tile([P, 8, 1024], dt.bfloat16)
            for cb in range(8):
                nc.sync.dma_start(
                    wo_sb[:, cb, :].rearrange("p (a b) -> p a b", a=2),
                    wo_d[cb].rearrange("a p b -> p a b"))
            scr2_p = d_es.enter_context(tc.tile_pool(name="scrD", bufs=2, side="right"))
            st2_p = d_es.enter_context(tc.tile_pool(name="stD", bufs=3, side="right"))

            bo = opt_sb.get("bo")
            for mb in range(8):
                for nb in range(2):
                    ps = ps_mm.tile([P, 512], dt.float32, tag="pmm")
                    for cb in range(8):
                        nc.tensor.matmul(
                            ps[:], ctxT[:, cb, mb * P:(mb + 1) * P],
                            wo_sb[:, cb, nb * 512:(nb + 1) * 512],
                            start=(cb == 0), stop=(cb == 7))
                    dst = attn_out[:, mb, nb * 512:(nb + 1) * 512]
                    xs = x_own[:, mb, nb * 512:(nb + 1) * 512]
                    if bo is None:
                        nc.vector.tensor_tensor(dst, ps[:], xs, op=OP.add)
                    else:
                        tmp = scr2_p.tile([P, 512], dt.float32, tag="botmp")
                        nc.vector.tensor_tensor(tmp[:], ps[:], xs, op=OP.add)
                        nc.vector.tensor_tensor(
                            dst, tmp[:], bo[:, nb * 512:(nb + 1) * 512], op=OP.add)

            ln2_w = opt_sb.get("anw")
            ln2_b = opt_sb.get("anb")
            for mb in range(8):
                stats = st2_p.tile([P, 12], dt.float32, tag="stats2")
                mv = st2_p.tile([P, 2], dt.float32, tag="mv2")
                nc.vector.bn_stats(stats[:, 0:6], attn_out[:, mb, 0:512])
                nc.vector.bn_stats(stats[:, 6:12], attn_out[:, mb, 512:1024])
                nc.vector.bn_aggr(mv[:], stats[:])
                sig = st2_p.tile([P, 1], dt.float32, tag="sig2")
                nc.scalar.activation(sig[:], mv[:, 1:2], AF.Sqrt, bias=eps_c[:])
                rsig = st2_p.tile([P, 1], dt.float32, tag="rsig2")
                nc.vector.reciprocal(rsig[:], sig[:])
                nbias = st2_p.tile([P, 1], dt.float32, tag="nbias2")
                nc.vector.scalar_tensor_tensor(
                    nbias[:], mv[:, 0:1], -1.0, rsig[:], OP.mult, OP.mult)
                yln = scr2_p.tile([P, H], dt.bfloat16, tag="yln")
                if ln2_w is None:
                    nc.scalar.activation(yln[:], attn_out[:, mb, :], AF.Identity,
                                         bias=nbias[:], scale=rsig[:])
                else:
                    tmp = st2_p.tile([P, H], dt.float32, tag="lntmp2")
                    nc.scalar.activation(tmp[:], attn_out[:, mb, :], AF.Identity,
                                         bias=nbias[:], scale=rsig[:])
                    if ln2_b is None:
                        nc.vector.tensor_tensor(yln[:], tmp[:], ln2_w[:], op=OP.mult)
                    else:
                        nc.vector.tensor_tensor(tmp[:], tmp[:], ln2_w[:], op=OP.mult)
                        nc.vector.tensor_tensor(yln[:], tmp[:], ln2_b[:], op=OP.add)
                transpose_1024(yln, ylnT, mb * P, ps_tr)

            d_es.close()     # free wo + stage-D scratch (right)
            ctx_es.close()   # left: ctxT
            qkv_es.close()   # left: vplus, kT, qT
            x_es.close()     # left: x_own
            # ================= stage E: FFN ==============================
            e_es = contextlib.ExitStack()
            hT_p = e_es.enter_context(tc.tile_pool(name="hT", bufs=1, side="left"))
            hT = hT_p.tile([P, 32, OWN], dt.bfloat16)
            w2_p = e_es.enter_context(tc.tile_pool(name="w2", bufs=1, side="left"))
            w2_sb = w2_p.tile([P, 32, 1024], dt.bfloat16)
            w1_p = e_es.enter_context(tc.tile_pool(name="w1", bufs=3, side="right"))
            for fb in range(32):
                nc.sync.dma_start(
                    w2_sb[:, fb, :].rearrange("p (a b) -> p a b", a=2),
                    w2_d[fb].rearrange("a p b -> p a b"))
            out_p = e_es.enter_context(tc.tile_pool(name="outp", bufs=2, side="right"))

            b1 = opt_sb.get("b1")
            for fb in range(32):
                w1_t = w1_p.tile([P, 8, P], dt.bfloat16, tag="w1t")
                nc.sync.dma_start(w1_t[:], w1_d[fb].rearrange("a p b -> p a b"))
                for qb in range(2):
                    ps = ps_mm.tile([P, 512], dt.float32, tag="pmm")
                    for hb in range(8):
                        nc.tensor.matmul(
                            ps[:], w1_t[:, hb, :],
                            ylnT[:, hb, qb * 512:(qb + 1) * 512],
                            start=(hb == 0), stop=(hb == 7))
                    bias = b1[:, fb, :] if b1 is not None else zero_c[:]
                    nc.scalar.activation(hT[:, fb, qb * 512:(qb + 1) * 512],
                                         ps[:], AF.Gelu, bias=bias)

            b2 = opt_sb.get("b2")
            for mb in range(8):
                outt = out_p.tile([P, H], dt.float32, tag="outt")
                for nb in range(2):
                    ps = ps_mm.tile([P, 512], dt.float32, tag="pmm")
                    for fb in range(32):
                        nc.tensor.matmul(
                            ps[:], hT[:, fb, mb * P:(mb + 1) * P],
                            w2_sb[:, fb, nb * 512:(nb + 1) * 512],
                            start=(fb == 0), stop=(fb == 31))
                    dst = outt[:, nb * 512:(nb + 1) * 512]
                    res = attn_out[:, mb, nb * 512:(nb + 1) * 512]
                    if b2 is None:
                        nc.vector.tensor_tensor(dst, ps[:], res, op=OP.add)
                    else:
                        nc.vector.tensor_tensor(dst, ps[:], res, op=OP.add)
                        nc.vector.tensor_tensor(
                            dst, dst, b2[:, nb * 512:(nb + 1) * 512], op=OP.add)
                nc.sync.dma_start(out_d[mb], outt[:])

            e_es.close()
            dkeep_es.close()

    nc.compile()
    return nc


# ---------------------------------------------------------------- host side
def _prep_weights(qkv_w, attn_ow, inter_w, output_w):
    bf = ml_dtypes.bfloat16
    wq = qkv_w[:, 0:1024].reshape(8, P, 8, P).transpose(2, 0, 1, 3).astype(bf)
    wk = qkv_w[:, 1024:2048].reshape(8, P, 8, P).transpose(2, 0, 1, 3).astype(bf)
    wv = qkv_w[:, 2048:3072].reshape(8, P, 2, 512).transpose(2, 0, 1, 3).astype(bf)
    wo = attn_ow.reshape(8, P, 2, 512).transpose(0, 2, 1, 3).astype(bf)
    w1 = inter_w.reshape(8, P, 32, P).transpose(2, 0, 1, 3).astype(bf)
    w2 = output_w.reshape(32, P, 2, 512).transpose(0, 2, 1, 3).astype(bf)
    return (np.ascontiguousarray(wq), np.ascontiguousarray(wk),
            np.ascontiguousarray(wv), np.ascontiguousarray(wo),
            np.ascontiguousarray(w1), np.ascontiguousarray(w2))


def kernel(input, input_mask, norm_w, norm_b, qkv_w, qkv_b, attn_ow, attn_ob,
           attn_nw, attn_nb, inter_w, inter_b, output_w, output_b):
    global LAST_EXEC_NS
    _install_trace_hook()
    from concourse.bass_utils import run_bass_kernel_spmd

    input = np.asarray(input, dtype=np.float32)
    input_mask = np.asarray(input_mask, dtype=np.float32)
    f32 = lambda a: np.asarray(a, dtype=np.float32)
    norm_w, norm_b = f32(norm_w), f32(norm_b)
    qkv_b, attn_ob = f32(qkv_b), f32(attn_ob)
    attn_nw, attn_nb = f32(attn_nw), f32(attn_nb)
    inter_b, output_b = f32(inter_b), f32(output_b)

    flags = {
        "ln1": not (np.all(norm_w == 1.0) and np.all(norm_b == 0.0)),
        "ln2": not (np.all(attn_nw == 1.0) and np.all(attn_nb == 0.0)),
        "bqk": bool(np.any(qkv_b[0:2048] != 0.0)),
        "bv": bool(np.any(qkv_b[2048:3072] != 0.0)),
        "bo": bool(np.any(attn_ob != 0.0)),
        "b1": bool(np.any(inter_b != 0.0)),
        "b2": bool(np.any(output_b != 0.0)),
        "mask": bool(np.any(input_mask != 0.0)),
    }
    key = tuple(sorted(flags.items()))
    if key not in _CACHE:
        _CACHE[key] = _build(flags)
    nc = _CACHE[key]

    wq, wk, wv, wo, w1, w2 = _prep_weights(
        f32(qkv_w), f32(attn_ow), f32(inter_w), f32(output_w))

    common = {"wq": wq, "wk": wk, "wv": wv, "wo": wo, "w1": w1, "w2": w2}
    bcast = lambda v: np.ascontiguousarray(
        np.broadcast_to(v.reshape(1, H), (P, H)).astype(np.float32))
    if flags["ln1"]:
        common["nw"] = bcast(norm_w); common["nb"] = bcast(norm_b)
    if flags["ln2"]:
        common["anw"] = bcast(attn_nw); common["anb"] = bcast(attn_nb)
    if flags["bqk"]:
        common["bqk"] = np.ascontiguousarray(
            qkv_b[0:2048].reshape(16, P, 1).astype(np.float32))
    if flags["bv"]:
        common["bv"] = np.ascontiguousarray(
            qkv_b[2048:3072].reshape(2, 512)[:, None, :].repeat(P, 1).astype(np.float32))
    if flags["bo"]:
        common["bo"] = bcast(attn_ob)
    if flags["b1"]:
        common["b1"] = np.ascontiguousarray(
            inter_b.reshape(32, P, 1).astype(np.float32))
    if flags["b2"]:
        common["b2"] = bcast(output_b)

    in_maps = []
    for c in range(NCORES):
        b, r = c // 2, (c % 2) * OWN
        xb = np.roll(input[b], -r, axis=0) if r else input[b]
        m = dict(common)
        m["x"] = np.ascontiguousarray(xb.reshape(16, P, H))
        if flags["mask"]:
            mk = input_mask[b, 0, 0]
            mk = np.roll(mk, -r) if r else mk
            m["mask"] = np.ascontiguousarray(
                mk.reshape(16, P).T.astype(np.float32))
        in_maps.append(m)

    trace = bool(os.environ.get("BASS_TRACE"))
    res = run_bass_kernel_spmd(nc, in_maps, list(range(NCORES)), trace=trace)
    LAST_EXEC_NS = res.exec_time_ns
    if res.exec_time_ns is not None:
        print(f"HW exec time: {res.exec_time_ns} ns")

    out = np.empty((B, S, H), dtype=np.float32)
    for c in range(NCORES):
        b, r = c // 2, (c % 2) * OWN
        out[b, r:r + OWN] = res.results[c]["out"].reshape(OWN, H)
    return out
